# revision 5
# baseline (speedup 1.0000x reference)
"""Fused transformer block (LN -> QKV+RoPE -> attention -> out_proj) on 8
Trainium2 NeuronCores.

Sharding: batch (2-way) x heads (4-way) = 8 cores. Core c handles batch
b = c // 4 and the 4 heads starting at 4*(c%4). Each core produces the
out_proj partial sum over its 256 dh-dims; the host sums 4 partials per
batch and adds b_out.

Device math, per core (matmul inputs bf16, PSUM accum fp32):
- x passed transposed AND pre-cast to bf16 on host: xT [D, S].
- LN stats via TensorE ones-matmuls (sum_d x, sum_d x^2 over partitions).
- ln_g folded into the weights on host (weights bf16; wsum computed from
  the bf16-rounded weights so the mu-folding matches the matmul).
  Mean-centering folded into the QKV epilogue: u' = mu (x) wsum - Z
  (note the sign: u' = -u; q and k are both negated so scores are
  unchanged).
- RoPE in [e, s] layout; rotate-half swap via partition-sliced
  SBUF->SBUF DMAs; cos/sin host-precomputed. The per-position rstd is
  applied to the q side explicitly and folded into exp's per-partition
  scale on the k side; v applies rstd in its PSUM epilogue.
- scores^T[j,i] per (head, j-tile) with K=64; exp on ScalarE
  (scale = rstd_k[j]/8) writing bf16 probs; o^T accumulated over j with
  lhsT = [v | 1] (M=65, row 64 = softmax denominators for free).
- out_proj partial from o^T tiles -> HBM fp32; host reduces + adds b_out.
"""
import sys
sys.path.insert(0, "/opt/trn_rl_repo")
import numpy as np

B, S, D = 2, 2048, 1024
HEADS, HDIM = 16, 64
HALF = HDIM // 2
ROPE_THETA = 10000.0
N_CORES = 8
HPC = HEADS // 4            # heads per core = 4
EC = HPC * HDIM             # per-core q (or k, or v) width = 256
P = 128
NK = D // P                 # 8 d-tiles
NS = S // P                 # 16 s-tiles
VW = HDIM + 1               # v block width incl. ones column = 65

_cache = {}


def _build():
    import contextlib
    import concourse.bass as bass
    import concourse.bacc as bacc
    import concourse.tile as tile
    from concourse import mybir
    fp32 = mybir.dt.float32
    bf16 = mybir.dt.bfloat16
    OP = mybir.AluOpType
    AF = mybir.ActivationFunctionType

    nc = bacc.Bacc("TRN2", target_bir_lowering=False, debug=False,
                   enable_asserts=True, num_devices=N_CORES)

    xT = nc.dram_tensor("xT", [D, S], bf16, kind="ExternalInput").ap()
    wqkT = nc.dram_tensor("wqkT", [D, 2 * EC], bf16, kind="ExternalInput").ap()
    wvT = nc.dram_tensor("wvT", [D, EC], bf16, kind="ExternalInput").ap()
    woT = nc.dram_tensor("woT", [EC, D], bf16, kind="ExternalInput").ap()
    wsum_qk = nc.dram_tensor("wsum_qk", [2 * EC], fp32, kind="ExternalInput").ap()
    wvsum = nc.dram_tensor("wvsum", [EC], fp32, kind="ExternalInput").ap()
    cosf = nc.dram_tensor("cosf", [P, S], fp32, kind="ExternalInput").ap()
    sinsg = nc.dram_tensor("sinsg", [P, S], fp32, kind="ExternalInput").ap()
    out = nc.dram_tensor("out", [S, D], fp32, kind="ExternalOutput").ap()

    wqk_r = wqkT.rearrange("(k p) e -> p k e", p=P)

    with tile.TileContext(nc) as tc, contextlib.ExitStack() as ctx:
        singles = ctx.enter_context(tc.tile_pool(name="singles", bufs=1))
        dram_scr = ctx.enter_context(
            tc.tile_pool(name="dram_scr", bufs=1, space="DRAM"))
        qk_sb = singles.tile([P, 4, S], bf16)              # 16KB/part
        v_sb = singles.tile([P, NS, HPC * VW], bf16)       # 8.1KB/part
        nc.gpsimd.memset(v_sb[:], 1.0)
        rstdT = singles.tile([P, NS], fp32)
        muT = singles.tile([P, NS], fp32)
        rstdT8 = singles.tile([P, NS], fp32)
        onep = singles.tile([P, 2], fp32)
        nc.vector.memset(onep[:], 1.0)
        nc.vector.memset(onep[0:1, 1:2], 1e-5)
        eps_sb = onep[0:1, 1:2]
        ones16 = singles.tile([P, 1], bf16)
        nc.vector.memset(ones16[:], 1.0)
        ones_sb = ones16[:, 0:1]

        with tc.tile_pool(name="ph1a", bufs=1) as ph1a:
            xT_sb = ph1a.tile([P, NK, S], bf16)            # 32KB/part
            xT_r = xT.rearrange("(k p) s -> p k s", p=P)
            for k in range(NK):     # split across queues: 8 x 512KB
                eng = nc.sync if k % 2 == 0 else nc.gpsimd
                eng.dma_start(out=xT_sb[:, k, :], in_=xT_r[:, k, :])
            wsqk_sb = ph1a.tile([P, 4], fp32)
            nc.sync.dma_start(
                out=wsqk_sb[:],
                in_=bass.AP(tensor=wsum_qk.tensor, offset=wsum_qk.offset,
                            ap=[[1, P], [P, 4]]))
            mu_b = ph1a.tile([P, S], fp32)
            rstd_b = ph1a.tile([P, S], fp32)

            # ---------------- phase 0: LN stats ----------------
            with tc.tile_pool(name="p0ps_a", bufs=2, space="PSUM") as p0ps_a, \
                 tc.tile_pool(name="p0ps_b", bufs=1, space="PSUM") as p0ps_b, \
                 tc.tile_pool(name="p0scr", bufs=1) as p0scr, \
                 tc.tile_pool(name="p0tmp", bufs=3) as p0tmp:
                mu_sb = p0scr.tile([1, S], fp32)
                ssq_sb = p0scr.tile([1, S], fp32)
                rstd_sb = p0scr.tile([1, S], fp32)
                for c in range(4):
                    ps_sum = p0ps_a.tile([1, 512], fp32, tag="ps")
                    for k in range(NK):
                        nc.tensor.matmul(ps_sum[:], ones_sb,
                                         xT_sb[:, k, c * 512:(c + 1) * 512],
                                         start=(k == 0), stop=(k == NK - 1))
                    nc.scalar.mul(out=mu_sb[:, c * 512:(c + 1) * 512],
                                  in_=ps_sum[:], mul=1.0 / D)
                psq = [p0ps_b.tile([1, 512], fp32, tag=f"psq{c}", name=f"psq{c}")
                       for c in range(4)]
                for k in range(NK):
                    for h2 in range(2):
                        xsq = p0tmp.tile([P, S // 2], bf16, tag="xsq")
                        nc.vector.tensor_mul(
                            xsq[:], xT_sb[:, k, h2 * 1024:(h2 + 1) * 1024],
                            xT_sb[:, k, h2 * 1024:(h2 + 1) * 1024])
                        for c in range(2):
                            ci = h2 * 2 + c
                            nc.tensor.matmul(psq[ci][:], ones_sb,
                                             xsq[:, c * 512:(c + 1) * 512],
                                             start=(k == 0), stop=(k == NK - 1),
                                             skip_group_check=True)
                for c in range(4):
                    nc.vector.tensor_copy(out=ssq_sb[:, c * 512:(c + 1) * 512],
                                          in_=psq[c][:])
                # var = ssq/D - mu^2 ; rstd = 1/sqrt(var + eps)
                nc.vector.tensor_mul(rstd_sb[:], mu_sb[:], mu_sb[:])
                nc.vector.scalar_tensor_tensor(out=rstd_sb[:], in0=ssq_sb[:],
                                               scalar=1.0 / D, in1=rstd_sb[:],
                                               op0=OP.mult, op1=OP.subtract)
                nc.scalar.activation(rstd_sb[:], rstd_sb[:], AF.Sqrt,
                                     bias=eps_sb)
                nc.vector.reciprocal(out=rstd_sb[:], in_=rstd_sb[:])

                # SBUF->SBUF partition-broadcast is illegal; bounce via DRAM.
                mu_d = dram_scr.tile([1, S], fp32)
                rstd_d = dram_scr.tile([1, S], fp32)
                nc.sync.dma_start(out=mu_d[:], in_=mu_sb[:])
                nc.sync.dma_start(out=rstd_d[:], in_=rstd_sb[:])
                _md, _rd = mu_d[:], rstd_d[:]
                for hh in range(2):   # split broadcasts across queues
                    o0 = hh * (S // 2)
                    nc.sync.dma_start(
                        out=mu_b[:, o0:o0 + S // 2],
                        in_=bass.AP(tensor=_md.tensor,
                                    offset=_md.offset + o0,
                                    ap=[[0, P], [1, S // 2]]))
                    nc.gpsimd.dma_start(
                        out=rstd_b[:, o0:o0 + S // 2],
                        in_=bass.AP(tensor=_rd.tensor,
                                    offset=_rd.offset + o0,
                                    ap=[[0, P], [1, S // 2]]))
                # transposed per-s-tile scalars: [p, t] = vec[t*128 + p]
                nc.sync.dma_start(
                    out=rstdT[:],
                    in_=bass.AP(tensor=_rd.tensor, offset=_rd.offset,
                                ap=[[1, P], [P, NS]]))
                nc.sync.dma_start(
                    out=muT[:],
                    in_=bass.AP(tensor=_md.tensor, offset=_md.offset,
                                ap=[[1, P], [P, NS]]))
                nc.vector.tensor_scalar_mul(rstdT8[:], rstdT[:],
                                            float(HDIM) ** -0.5)

            # ---------------- phase 1a: Q/K matmuls + RoPE ----------------
            SH = S // 2
            with tc.tile_pool(name="ph1b", bufs=1) as ph1b, \
                 tc.tile_pool(name="p1w", bufs=2) as p1w, \
                 tc.tile_pool(name="p1psum", bufs=3, space="PSUM") as p1psum, \
                 tc.tile_pool(name="p1tmp", bufs=3) as p1tmp:
                cos_sb = ph1b.tile([P, S], fp32)
                sin_sb = ph1b.tile([P, S], fp32)
                for hh in range(2):
                    o0 = hh * (S // 2)
                    nc.sync.dma_start(out=cos_sb[:, o0:o0 + S // 2],
                                      in_=cosf[:, o0:o0 + S // 2])
                    nc.gpsimd.dma_start(out=sin_sb[:, o0:o0 + S // 2],
                                        in_=sinsg[:, o0:o0 + S // 2])
                for e in range(4):
                    wqk_e = p1w.tile([P, NK, P], bf16, tag="wqk")
                    nc.sync.dma_start(out=wqk_e[:],
                                      in_=wqk_r[:, :, e * P:(e + 1) * P])
                    for sh in range(2):
                        s0 = sh * SH
                        zq = p1psum.tile([P, SH], fp32, tag="zqk")
                        for c in range(2):
                            c0 = c * 512
                            for k in range(NK):
                                nc.tensor.matmul(
                                    zq[:, c0:c0 + 512],
                                    wqk_e[:, k, :],
                                    xT_sb[:, k, s0 + c0:s0 + c0 + 512],
                                    start=(k == 0), stop=(k == NK - 1))
                        # u' = mu (x) wsum - Z  (negated; sign cancels in qk)
                        u = p1tmp.tile([P, SH], fp32, tag="u")
                        nc.vector.scalar_tensor_tensor(
                            out=u[:], in0=mu_b[:, s0:s0 + SH],
                            scalar=wsqk_sb[:, e:e + 1], in1=zq[:],
                            op0=OP.mult, op1=OP.subtract)
                        # rotate-half swap (per 64-row head group)
                        usw = p1tmp.tile([P, SH], fp32, tag="usw")
                        for g in range(2):
                            b0 = g * HDIM
                            nc.sync.dma_start(out=usw[b0:b0 + HALF, :],
                                              in_=u[b0 + HALF:b0 + HDIM, :])
                            nc.sync.dma_start(out=usw[b0 + HALF:b0 + HDIM, :],
                                              in_=u[b0:b0 + HALF, :])
                        # rot = u*cos + usw*sin_signed (u dead after swap)
                        nc.vector.tensor_mul(u[:], u[:], cos_sb[:, s0:s0 + SH])
                        nc.vector.tensor_mul(usw[:], usw[:],
                                             sin_sb[:, s0:s0 + SH])
                        if e < 2:   # q side: multiply by rstd as well
                            nc.vector.tensor_add(u[:], u[:], usw[:])
                            nc.vector.tensor_mul(qk_sb[:, e, s0:s0 + SH],
                                                 u[:], rstd_b[:, s0:s0 + SH])
                        else:
                            nc.vector.tensor_add(qk_sb[:, e, s0:s0 + SH],
                                                 u[:], usw[:])

            # ---------------- phase 1b: V (natural layout) ----------------
            with tc.tile_pool(name="ph1c", bufs=1) as ph1c, \
                 tc.tile_pool(name="p1vps", bufs=3, space="PSUM") as p1vps, \
                 tc.tile_pool(name="p1vt", bufs=3) as p1vt:
                wv_sb = ph1c.tile([P, NK, EC], bf16)
                wv_r = wvT.rearrange("(k p) e -> p k e", p=P)
                for k in range(NK):
                    eng = nc.sync if k % 2 == 0 else nc.gpsimd
                    eng.dma_start(out=wv_sb[:, k, :], in_=wv_r[:, k, :])
                wsv_b = ph1c.tile([P, EC], fp32)
                nc.sync.dma_start(
                    out=wsv_b[:],
                    in_=bass.AP(tensor=wvsum.tensor, offset=wvsum.offset,
                                ap=[[0, P], [1, EC]]))
                for t in range(NS):
                    zv = p1vps.tile([P, EC], fp32, tag="zv")
                    for k in range(NK):
                        nc.tensor.matmul(zv[:], xT_sb[:, k, t * P:(t + 1) * P],
                                         wv_sb[:, k, :],
                                         start=(k == 0), stop=(k == NK - 1))
                    t2v = p1vt.tile([P, EC], fp32, tag="t2v")
                    nc.vector.tensor_scalar(out=t2v[:], in0=wsv_b[:],
                                            scalar1=muT[:, t:t + 1],
                                            scalar2=rstdT[:, t:t + 1],
                                            op0=OP.mult, op1=OP.mult)
                    # v = rstd_s * Zv - (mu*rstd)_s * wvsum
                    nc.vector.scalar_tensor_tensor(
                        out=v_sb[:, t, :].rearrange("p (h w) -> p h w",
                                                    h=HPC)[:, :, 0:HDIM],
                        in0=zv[:].rearrange("p (h d) -> p h d", h=HPC),
                        scalar=rstdT[:, t:t + 1],
                        in1=t2v[:].rearrange("p (h d) -> p h d", h=HPC),
                        op0=OP.mult, op1=OP.subtract)

        # ---------------- phase 2: attention ----------------
        import os
        _abl = os.environ.get("ABLATE", "")
        if _abl == "p01":
            nc.compile()
            return nc
        with tc.tile_pool(name="late", bufs=1) as late:
          oT_sb = late.tile([P, 2, S], bf16)       # o^T (4 heads x 64 rows)
          with tc.tile_pool(name="ps_s", bufs=2, space="PSUM") as ps_s, \
               tc.tile_pool(name="ps_o", bufs=1, space="PSUM") as ps_o, \
               tc.tile_pool(name="p2tmp", bufs=3) as p2tmp, \
               tc.tile_pool(name="p2rec", bufs=2) as p2rec, \
               tc.tile_pool(name="p2recd", bufs=2, space="DRAM") as p2recd:
            for h in range(HPC):
                et = h // 2
                ep = (h % 2) * HDIM
                po = ps_o.tile([VW, S], fp32, tag="po")
                for j in range(NS):
                    p_sb = p2tmp.tile([P, S], bf16, tag="p")
                    for sh in range(2):
                        pscore = ps_s.tile([P, S // 2], fp32, tag="ps")
                        for c in range(2):
                            c0 = c * 512
                            nc.tensor.matmul(
                                pscore[:, c0:c0 + 512],
                                qk_sb[ep:ep + HDIM, 2 + et, j * P:(j + 1) * P],
                                qk_sb[ep:ep + HDIM, et,
                                      sh * 1024 + c0:sh * 1024 + c0 + 512],
                                start=True, stop=True)
                        nc.scalar.activation(p_sb[:, sh * 1024:(sh + 1) * 1024],
                                             pscore[:], AF.Exp,
                                             scale=rstdT8[:, j:j + 1])
                    for c in range(4):
                        nc.tensor.matmul(po[:, c * 512:(c + 1) * 512],
                                         v_sb[:, j, h * VW:(h + 1) * VW],
                                         p_sb[:, c * 512:(c + 1) * 512],
                                         start=(j == 0), stop=(j == NS - 1),
                                         skip_group_check=True)
                rec = p2rec.tile([1, S], fp32, tag="rec")
                nc.vector.reciprocal(out=rec[:], in_=po[HDIM:HDIM + 1, :])
                rec_d = p2recd.tile([1, S], fp32, tag="recd", name="rec_d")
                nc.sync.dma_start(out=rec_d[:], in_=rec[:])
                recb = p2rec.tile([HDIM, S], fp32, tag="recb")
                _rc = rec_d[:]
                for hh in range(2):
                    o0 = hh * (S // 2)
                    eng = nc.sync if hh == 0 else nc.gpsimd
                    eng.dma_start(
                        out=recb[:, o0:o0 + S // 2],
                        in_=bass.AP(tensor=_rc.tensor,
                                    offset=_rc.offset + o0,
                                    ap=[[0, HDIM], [1, S // 2]]))
                nc.vector.tensor_mul(oT_sb[ep:ep + HDIM, et, :],
                                     po[0:HDIM, :], recb[:])

          # ---------------- phase 3: out_proj partial ----------------
          with tc.tile_pool(name="p3w", bufs=1) as p3w, \
               tc.tile_pool(name="p3psum", bufs=2, space="PSUM") as p3psum, \
               tc.tile_pool(name="p3tmp", bufs=3) as p3tmp:
                wo_sb = p3w.tile([P, 2, D], bf16)
                wo_r = woT.rearrange("(k p) e -> p k e", p=P)
                for k in range(2):
                    eng = nc.sync if k % 2 == 0 else nc.gpsimd
                    eng.dma_start(out=wo_sb[:, k, :], in_=wo_r[:, k, :])
                for t in range(NS):
                    pout = p3psum.tile([P, D], fp32, tag="pout")
                    for c in range(2):
                        for k in range(2):
                            nc.tensor.matmul(pout[:, c * 512:(c + 1) * 512],
                                             oT_sb[:, k, t * P:(t + 1) * P],
                                             wo_sb[:, k, c * 512:(c + 1) * 512],
                                             start=(k == 0), stop=(k == 1))
                    ot = p3tmp.tile([P, D], fp32, tag="ot")
                    nc.vector.tensor_copy(out=ot[:], in_=pout[:])
                    eng = nc.sync if t % 2 == 0 else nc.gpsimd
                    eng.dma_start(out=out[t * P:(t + 1) * P, :], in_=ot[:])

    nc.compile()
    return nc


def _host_inputs(x, ln_g, ln_b, w_qkv, w_out):
    import ml_dtypes
    bf16 = ml_dtypes.bfloat16
    wq = w_qkv[0:D] * ln_g[None, :]
    wk = w_qkv[D:2 * D] * ln_g[None, :]
    wv = w_qkv[2 * D:3 * D] * ln_g[None, :]
    if np.abs(w_qkv.astype(np.float32) @ ln_b.astype(np.float32)).max() != 0.0:
        raise NotImplementedError("nonzero ln_b not supported")
    inv = 1.0 / (ROPE_THETA ** (np.arange(0, HALF, dtype=np.float32) / HALF))
    fr = np.arange(S, dtype=np.float32)[:, None] * inv[None, :]
    cos = np.cos(fr).T.astype(np.float32)          # [32, S]
    sin = np.sin(fr).T.astype(np.float32)
    # row layout per 64-group: [lo(32); hi(32)]; cos same both halves.
    cosf = np.tile(cos, (4, 1))                    # [128, S]
    # rot_lo = lo*c - hi*s ; rot_hi = hi*c + lo*s. usw = [hi; lo], so the
    # sin multiplier rows are [-s (for lo out); +s (for hi out)].
    sinsg = np.tile(np.concatenate([-sin, sin], 0), (2, 1))
    ins = []
    for core in range(N_CORES):
        b = core // 4
        h0 = (core % 4) * HPC
        sl = slice(h0 * HDIM, (h0 + HPC) * HDIM)
        wq_c, wk_c, wv_c = wq[sl], wk[sl], wv[sl]
        qk16 = np.concatenate([wq_c, wk_c], 0).astype(bf16)
        wv16 = wv_c.astype(bf16)
        ins.append({
            "xT": np.ascontiguousarray(x[b].T.astype(bf16)),
            "wqkT": np.ascontiguousarray(qk16.T),
            "wvT": np.ascontiguousarray(wv16.T),
            "woT": np.ascontiguousarray(w_out[:, sl].T.astype(bf16)),
            # wsum from the bf16-rounded weights so mu-folding matches
            "wsum_qk": qk16.astype(np.float32).sum(1),
            "wvsum": wv16.astype(np.float32).sum(1),
            "cosf": cosf, "sinsg": sinsg,
        })
    return ins


def kernel(x, ln_g, ln_b, w_qkv, w_out, b_out):
    from concourse import bass_utils
    x = np.asarray(x, np.float32)
    ln_g = np.asarray(ln_g, np.float32)
    ln_b = np.asarray(ln_b, np.float32)
    w_qkv = np.asarray(w_qkv, np.float32)
    w_out = np.asarray(w_out, np.float32)
    b_out = np.asarray(b_out, np.float32)
    if "nc" not in _cache:
        _cache["nc"] = _build()
    ins = _host_inputs(x, ln_g, ln_b, w_qkv, w_out)
    res = bass_utils.run_bass_kernel_spmd(_cache["nc"], ins,
                                          core_ids=list(range(N_CORES)))
    _cache["last_results"] = res
    out = np.zeros((B, S, D), np.float32)
    for core in range(N_CORES):
        out[core // 4] += res.results[core]["out"]
    out += b_out[None, None, :]
    return out


# revision 6
# speedup vs baseline: 4.3699x; 4.3699x over previous
"""Fused transformer block (LN -> QKV+RoPE -> attention -> out_proj) on 8
Trainium2 NeuronCores.

Sharding: batch (2-way) x heads (4-way) = 8 cores. Core c handles batch
b = c // 4 and the 4 heads starting at 4*(c%4). Each core produces the
out_proj partial sum over its 256 dh-dims; the host sums 4 partials per
batch and adds b_out.

Device math, per core (matmul inputs bf16, PSUM accum fp32):
- x passed transposed AND pre-cast to bf16 on host: xT [D, S].
- LN stats via TensorE ones-matmuls (sum_d x, sum_d x^2 over partitions),
  one fused k-loop so PE chases the xT DMA.
- ln_g folded into the weights on host (weights bf16; wsum computed from
  the bf16-rounded weights so the mu-folding matches the matmul).
- Mean-centering folded into the matmul epilogue ON PE: a K=1 matmul
  with lhsT = -wsum (bf16) and rhs = mu (bf16) accumulates -mu*wsum into
  the same PSUM group, so u = Z - mu*wsum comes out of PSUM directly.
- RoPE in [e, s] layout; u copied PSUM->SBUF bf16, rotate-half swap via
  cheap bf16 partition-sliced SBUF->SBUF DMAs, then all-bf16 DVE muls
  with host-precomputed cos/sin (sin carries the rotate-half signs).
  Per-position rstd: applied to q explicitly (bf16 broadcast), folded
  into exp's per-partition scale on the k side; v applies rstd in its
  PSUM epilogue.
- attention per (head, i-half): scores^T[j,i] K=64 one N=1024 matmul;
  exp on ScalarE (scale = rstd_k[j]/8) -> bf16 probs; o^T accumulated
  over j with lhsT = [v | 1] (M=65, row 64 = softmax denominators).
  Software-pipelined emission (scores_{j+1} before AV_j) keeps the PE
  FIFO from head-blocking on the exp; po double-buffered (i-halved) so
  the denominator/reciprocal chain never stalls the next accumulation.
- out_proj partial from o^T tiles; PSUM->SBUF copies on ScalarE (idle
  after the last exp); DMA to HBM fp32; host reduces + adds b_out.
"""
import sys
sys.path.insert(0, "/opt/trn_rl_repo")
import numpy as np

B, S, D = 2, 2048, 1024
HEADS, HDIM = 16, 64
HALF = HDIM // 2
ROPE_THETA = 10000.0
N_CORES = 8
HPC = HEADS // 4            # heads per core = 4
EC = HPC * HDIM             # per-core q (or k, or v) width = 256
P = 128
NK = D // P                 # 8 d-tiles
NS = S // P                 # 16 s-tiles
SH = S // 2                 # i-half width = 1024
VW = HDIM + 1               # v block width incl. ones column = 65

_cache = {}


def _build():
    import contextlib
    import concourse.bass as bass
    import concourse.bacc as bacc
    import concourse.tile as tile
    from concourse import mybir
    fp32 = mybir.dt.float32
    bf16 = mybir.dt.bfloat16
    OP = mybir.AluOpType
    AF = mybir.ActivationFunctionType

    nc = bacc.Bacc("TRN2", target_bir_lowering=False, debug=False,
                   enable_asserts=True, num_devices=N_CORES)

    xT = nc.dram_tensor("xT", [D, S], bf16, kind="ExternalInput").ap()
    wqkT = nc.dram_tensor("wqkT", [D, 2 * EC], bf16, kind="ExternalInput").ap()
    wvT = nc.dram_tensor("wvT", [D, EC], bf16, kind="ExternalInput").ap()
    woT = nc.dram_tensor("woT", [EC, D], bf16, kind="ExternalInput").ap()
    nws_qk = nc.dram_tensor("nws_qk", [1, 2 * EC], bf16,
                            kind="ExternalInput").ap()
    nws_v = nc.dram_tensor("nws_v", [1, EC], bf16, kind="ExternalInput").ap()
    cosf = nc.dram_tensor("cosf", [P, S], bf16, kind="ExternalInput").ap()
    sinsg = nc.dram_tensor("sinsg", [P, S], bf16, kind="ExternalInput").ap()
    out = nc.dram_tensor("out", [S, D], fp32, kind="ExternalOutput").ap()

    with tile.TileContext(nc) as tc, contextlib.ExitStack() as ctx:
        singles = ctx.enter_context(tc.tile_pool(name="singles", bufs=1))
        dram_scr = ctx.enter_context(
            tc.tile_pool(name="dram_scr", bufs=1, space="DRAM"))
        qk_sb = singles.tile([P, 4, S], bf16)              # 16KB/part
        v_sb = singles.tile([P, NS, HPC * VW], bf16)       # 8.1KB/part
        nc.gpsimd.memset(v_sb[:], 1.0)
        rstdT = singles.tile([P, NS], fp32)
        rstdT8 = singles.tile([P, NS], fp32)
        onep = singles.tile([P, 2], fp32)
        nc.vector.memset(onep[:], 1.0)
        nc.vector.memset(onep[0:1, 1:2], 1e-5)
        eps_sb = onep[0:1, 1:2]
        ones16 = singles.tile([P, 1], bf16)
        nc.vector.memset(ones16[:], 1.0)
        ones_sb = ones16[:, 0:1]
        mu16 = singles.tile([1, S], bf16)
        rstd16 = singles.tile([1, S], bf16)
        rstd16_b = singles.tile([P, S], bf16)
        nws_sb = singles.tile([1, 2 * EC], bf16)
        nwsv_sb = singles.tile([1, EC], bf16)
        cos_sb = singles.tile([P, S], bf16)
        sin_sb = singles.tile([P, S], bf16)
        wo_sb = singles.tile([P, 2, D], bf16)
        oT_sb = singles.tile([P, 2, S], bf16)       # o^T (4 heads x 64 rows)

        with tc.tile_pool(name="ph1a", bufs=1) as ph1a:
            xT_sb = ph1a.tile([P, NK, S], bf16)            # 32KB/part
            wqk_sb = ph1a.tile([P, NK, 4 * P], bf16)       # 8KB/part
            wv_sb = ph1a.tile([P, NK, EC], bf16)           # 4KB/part
            xT_r = xT.rearrange("(k p) s -> p k s", p=P)
            wqk_r = wqkT.rearrange("(k p) e -> p k e", p=P)
            wv_r = wvT.rearrange("(k p) e -> p k e", p=P)
            wo_r = woT.rearrange("(k p) e -> p k e", p=P)
            # prefetch everything on the sync queue; x k0 first so LN
            # stats start immediately, weights next so no Ldweights stall.
            nc.sync.dma_start(out=xT_sb[:, 0, :], in_=xT_r[:, 0, :])
            nc.sync.dma_start(out=wqk_sb[:], in_=wqk_r[:])
            for k in range(1, NK):
                nc.sync.dma_start(out=xT_sb[:, k, :], in_=xT_r[:, k, :])
            nc.sync.dma_start(out=wv_sb[:], in_=wv_r[:])
            nc.sync.dma_start(out=wo_sb[:], in_=wo_r[:])
            nc.sync.dma_start(out=cos_sb[:], in_=cosf[:])
            nc.sync.dma_start(out=sin_sb[:], in_=sinsg[:])
            nc.sync.dma_start(out=nws_sb[:], in_=nws_qk[:])
            nc.sync.dma_start(out=nwsv_sb[:], in_=nws_v[:])

            # ---------------- phase 0: LN stats ----------------
            with tc.tile_pool(name="p0ps_a", bufs=1, space="PSUM") as p0ps_a, \
                 tc.tile_pool(name="p0ps_b", bufs=1, space="PSUM") as p0ps_b, \
                 tc.tile_pool(name="p0scr", bufs=1) as p0scr, \
                 tc.tile_pool(name="p0tmp", bufs=3) as p0tmp:
                rstd_sb = p0scr.tile([1, S], fp32)
                pss = [p0ps_a.tile([1, 512], fp32, tag=f"pss{c}",
                                   name=f"pss{c}") for c in range(4)]
                psq = [p0ps_b.tile([1, 512], fp32, tag=f"psq{c}",
                                   name=f"psq{c}") for c in range(4)]
                for k in range(NK):
                    for c in range(4):
                        nc.tensor.matmul(pss[c][:], ones_sb,
                                         xT_sb[:, k, c * 512:(c + 1) * 512],
                                         start=(k == 0), stop=(k == NK - 1),
                                         skip_group_check=True)
                    for h2 in range(2):
                        xsq = p0tmp.tile([P, S // 2], bf16, tag="xsq")
                        nc.vector.tensor_mul(
                            xsq[:], xT_sb[:, k, h2 * 1024:(h2 + 1) * 1024],
                            xT_sb[:, k, h2 * 1024:(h2 + 1) * 1024])
                        for c in range(2):
                            ci = h2 * 2 + c
                            nc.tensor.matmul(psq[ci][:], ones_sb,
                                             xsq[:, c * 512:(c + 1) * 512],
                                             start=(k == 0), stop=(k == NK - 1),
                                             skip_group_check=True)
                for c in range(4):   # mu (bf16) straight from PSUM on Act
                    nc.scalar.mul(out=mu16[:, c * 512:(c + 1) * 512],
                                  in_=pss[c][:], mul=1.0 / D)
                # var = ssq/D - mu^2 ; rstd = 1/sqrt(var + eps)
                nc.vector.tensor_mul(rstd_sb[:], mu16[:], mu16[:])
                for c in range(4):
                    nc.vector.scalar_tensor_tensor(
                        out=rstd_sb[:, c * 512:(c + 1) * 512],
                        in0=psq[c][:], scalar=1.0 / D,
                        in1=rstd_sb[:, c * 512:(c + 1) * 512],
                        op0=OP.mult, op1=OP.subtract)
                nc.scalar.activation(rstd_sb[:], rstd_sb[:], AF.Sqrt,
                                     bias=eps_sb)
                nc.vector.reciprocal(out=rstd_sb[:], in_=rstd_sb[:])
                nc.scalar.mul(out=rstd16[:], in_=rstd_sb[:], mul=1.0)

                # SBUF->SBUF partition-broadcast is illegal; bounce via DRAM.
                rstd_d = dram_scr.tile([1, S], fp32)
                rstd16_d = dram_scr.tile([1, S], bf16)
                nc.sync.dma_start(out=rstd_d[:], in_=rstd_sb[:])
                nc.sync.dma_start(out=rstd16_d[:], in_=rstd16[:])
                _rd, _r6 = rstd_d[:], rstd16_d[:]
                nc.sync.dma_start(
                    out=rstd16_b[:],
                    in_=bass.AP(tensor=_r6.tensor, offset=_r6.offset,
                                ap=[[0, P], [1, S]]))
                # transposed per-s-tile scalars: [p, t] = vec[t*128 + p]
                nc.sync.dma_start(
                    out=rstdT[:],
                    in_=bass.AP(tensor=_rd.tensor, offset=_rd.offset,
                                ap=[[1, P], [P, NS]]))
                nc.vector.tensor_scalar_mul(rstdT8[:], rstdT[:],
                                            float(HDIM) ** -0.5)

            # ---------------- phase 1a: Q/K matmuls + RoPE ----------------
            # k-side (e=2,3) first: it doesn't need rstd16_b yet.
            with tc.tile_pool(name="p1psum", bufs=3, space="PSUM") as p1psum, \
                 tc.tile_pool(name="p1tmp", bufs=3) as p1tmp:
                for e in (2, 3, 0, 1):
                    for sh in range(2):
                        s0 = sh * SH
                        zq = p1psum.tile([P, SH], fp32, tag="zqk")
                        for k in range(NK):
                            nc.tensor.matmul(
                                zq[:], wqk_sb[:, k, e * P:(e + 1) * P],
                                xT_sb[:, k, s0:s0 + SH],
                                start=(k == 0), stop=False)
                        # u = Z - mu*wsum via K=1 matmul (mu as moving row)
                        nc.tensor.matmul(
                            zq[:], nws_sb[:, e * P:(e + 1) * P],
                            mu16[:, s0:s0 + SH], start=False, stop=True)
                        u = p1tmp.tile([P, SH], bf16, tag="u")
                        nc.vector.tensor_copy(out=u[:], in_=zq[:])
                        # rotate-half swap (per 64-row head group)
                        usw = p1tmp.tile([P, SH], bf16, tag="usw")
                        for g in range(2):
                            b0 = g * HDIM
                            nc.sync.dma_start(out=usw[b0:b0 + HALF, :],
                                              in_=u[b0 + HALF:b0 + HDIM, :])
                            nc.sync.dma_start(out=usw[b0 + HALF:b0 + HDIM, :],
                                              in_=u[b0:b0 + HALF, :])
                        # rot = u*cos + usw*sin_signed (all-bf16 SBUF DVE)
                        nc.vector.tensor_mul(u[:], u[:], cos_sb[:, s0:s0 + SH])
                        nc.vector.tensor_mul(usw[:], usw[:],
                                             sin_sb[:, s0:s0 + SH])
                        if e < 2:   # q side: multiply by rstd as well
                            nc.vector.tensor_add(u[:], u[:], usw[:])
                            nc.vector.tensor_mul(qk_sb[:, e, s0:s0 + SH],
                                                 u[:],
                                                 rstd16_b[:, s0:s0 + SH])
                        else:
                            nc.vector.tensor_add(qk_sb[:, e, s0:s0 + SH],
                                                 u[:], usw[:])

            # ---------------- phase 1b: V (natural layout) ----------------
            with tc.tile_pool(name="p1vps", bufs=3, space="PSUM") as p1vps:
                for t in range(NS):
                    zv = p1vps.tile([P, EC], fp32, tag="zv")
                    for k in range(NK):
                        nc.tensor.matmul(zv[:], xT_sb[:, k, t * P:(t + 1) * P],
                                         wv_sb[:, k, :],
                                         start=(k == 0), stop=False)
                    nc.tensor.matmul(zv[:], mu16[:, t * P:(t + 1) * P],
                                     nwsv_sb[:], start=False, stop=True)
                    # v = rstd_s * (Zv - mu*wsum)
                    nc.vector.tensor_scalar_mul(
                        v_sb[:, t, :].rearrange("p (h w) -> p h w",
                                                h=HPC)[:, :, 0:HDIM],
                        zv[:].rearrange("p (h d) -> p h d", h=HPC),
                        rstdT[:, t:t + 1])

        # ---------------- phase 2: attention ----------------
        with tc.tile_pool(name="ps_s", bufs=2, space="PSUM") as ps_s, \
             tc.tile_pool(name="ps_o", bufs=2, space="PSUM") as ps_o, \
             tc.tile_pool(name="p2tmp", bufs=3) as p2tmp, \
             tc.tile_pool(name="p2rec", bufs=2) as p2rec, \
             tc.tile_pool(name="p2recd", bufs=2, space="DRAM") as p2recd:
            for h in range(HPC):
                et = h // 2
                ep = (h % 2) * HDIM
                for ih in range(2):
                    i0 = ih * SH
                    po = ps_o.tile([VW, SH], fp32, tag="po")
                    p_prev = None
                    for j in range(NS):
                        pscore = ps_s.tile([P, SH], fp32, tag="ps")
                        nc.tensor.matmul(
                            pscore[:],
                            qk_sb[ep:ep + HDIM, 2 + et, j * P:(j + 1) * P],
                            qk_sb[ep:ep + HDIM, et, i0:i0 + SH],
                            start=True, stop=True)
                        p_sb = p2tmp.tile([P, SH], bf16, tag="p")
                        nc.scalar.activation(p_sb[:], pscore[:], AF.Exp,
                                             scale=rstdT8[:, j:j + 1])
                        if p_prev is not None:
                            nc.tensor.matmul(po[:],
                                             v_sb[:, j - 1,
                                                  h * VW:(h + 1) * VW],
                                             p_prev[:],
                                             start=(j == 1), stop=False,
                                             skip_group_check=True)
                        p_prev = p_sb
                    nc.tensor.matmul(po[:],
                                     v_sb[:, NS - 1, h * VW:(h + 1) * VW],
                                     p_prev[:], start=False, stop=True,
                                     skip_group_check=True)
                    rec = p2rec.tile([1, SH], fp32, tag="rec")
                    nc.vector.reciprocal(out=rec[:], in_=po[HDIM:HDIM + 1, :])
                    rec_d = p2recd.tile([1, SH], fp32, tag="recd",
                                        name="rec_d")
                    nc.sync.dma_start(out=rec_d[:], in_=rec[:])
                    recb = p2rec.tile([HDIM, SH], fp32, tag="recb")
                    _rc = rec_d[:]
                    nc.sync.dma_start(
                        out=recb[:],
                        in_=bass.AP(tensor=_rc.tensor, offset=_rc.offset,
                                    ap=[[0, HDIM], [1, SH]]))
                    nc.vector.tensor_mul(oT_sb[ep:ep + HDIM, et, i0:i0 + SH],
                                         po[0:HDIM, :], recb[:])

        # ---------------- phase 3: out_proj partial ----------------
        with tc.tile_pool(name="p3psum", bufs=2, space="PSUM") as p3psum, \
             tc.tile_pool(name="p3tmp", bufs=3) as p3tmp:
            for t in range(NS):
                pout = p3psum.tile([P, D], fp32, tag="pout")
                for k in range(2):
                    nc.tensor.matmul(pout[:], oT_sb[:, k, t * P:(t + 1) * P],
                                     wo_sb[:, k, :],
                                     start=(k == 0), stop=(k == 1))
                ot = p3tmp.tile([P, D], fp32, tag="ot")
                nc.scalar.mul(out=ot[:], in_=pout[:], mul=1.0)
                nc.sync.dma_start(out=out[t * P:(t + 1) * P, :], in_=ot[:])

    nc.compile()
    return nc


def _host_inputs(x, ln_g, ln_b, w_qkv, w_out):
    import ml_dtypes
    bf16 = ml_dtypes.bfloat16
    wq = w_qkv[0:D] * ln_g[None, :]
    wk = w_qkv[D:2 * D] * ln_g[None, :]
    wv = w_qkv[2 * D:3 * D] * ln_g[None, :]
    if np.abs(w_qkv.astype(np.float32) @ ln_b.astype(np.float32)).max() != 0.0:
        raise NotImplementedError("nonzero ln_b not supported")
    inv = 1.0 / (ROPE_THETA ** (np.arange(0, HALF, dtype=np.float32) / HALF))
    fr = np.arange(S, dtype=np.float32)[:, None] * inv[None, :]
    cos = np.cos(fr).T.astype(np.float32)          # [32, S]
    sin = np.sin(fr).T.astype(np.float32)
    # row layout per 64-group: [lo(32); hi(32)]; cos same both halves.
    cosf = np.tile(cos, (4, 1)).astype(bf16)       # [128, S]
    # rot_lo = lo*c - hi*s ; rot_hi = hi*c + lo*s. usw = [hi; lo], so the
    # sin multiplier rows are [-s (for lo out); +s (for hi out)].
    sinsg = np.tile(np.concatenate([-sin, sin], 0), (2, 1)).astype(bf16)
    ins = []
    for core in range(N_CORES):
        b = core // 4
        h0 = (core % 4) * HPC
        sl = slice(h0 * HDIM, (h0 + HPC) * HDIM)
        wq_c, wk_c, wv_c = wq[sl], wk[sl], wv[sl]
        qk16 = np.concatenate([wq_c, wk_c], 0).astype(bf16)
        wv16 = wv_c.astype(bf16)
        # wsum from the bf16-rounded weights so mu-folding matches; negated
        # because the K=1 matmul ACCUMULATES -mu*wsum into Z.
        nws_qk = (-qk16.astype(np.float32).sum(1)).astype(bf16)[None, :]
        nws_v = (-wv16.astype(np.float32).sum(1)).astype(bf16)[None, :]
        ins.append({
            "xT": np.ascontiguousarray(x[b].T.astype(bf16)),
            "wqkT": np.ascontiguousarray(qk16.T),
            "wvT": np.ascontiguousarray(wv16.T),
            "woT": np.ascontiguousarray(w_out[:, sl].T.astype(bf16)),
            "nws_qk": nws_qk, "nws_v": nws_v,
            "cosf": cosf, "sinsg": sinsg,
        })
    return ins


def kernel(x, ln_g, ln_b, w_qkv, w_out, b_out):
    from concourse import bass_utils
    x = np.asarray(x, np.float32)
    ln_g = np.asarray(ln_g, np.float32)
    ln_b = np.asarray(ln_b, np.float32)
    w_qkv = np.asarray(w_qkv, np.float32)
    w_out = np.asarray(w_out, np.float32)
    b_out = np.asarray(b_out, np.float32)
    if "nc" not in _cache:
        _cache["nc"] = _build()
    ins = _host_inputs(x, ln_g, ln_b, w_qkv, w_out)
    res = bass_utils.run_bass_kernel_spmd(_cache["nc"], ins,
                                          core_ids=list(range(N_CORES)))
    _cache["last_results"] = res
    out = np.zeros((B, S, D), np.float32)
    for core in range(N_CORES):
        out[core // 4] += res.results[core]["out"]
    out += b_out[None, None, :]
    return out
